# revision 81
# baseline (speedup 1.0000x reference)
"""DRGRU (diffusion-conv GRU cell) Trainium2 kernel — fp8 DoubleRow edition.

Per-core (8 cores, one batch sample each):
  A0 = diag(1/colsum(adj+I)) @ (adj+I),  A1 = diag(1/rowsum(adj+I)) @ (adj+I)^T
  gconv(x) = [x, A0 x, A0 x, A0^2 x] @ W0^T + [x, A1 x, A1 x, A1^2 x] @ W1^T + b
  value = sigmoid(gconv1(cat(xi, xh)));  r,u = split(value)
  c = tanh(gconv2(cat(xi, r*xh)));  out = u*xh + (1-u)*c

Key layout/precision choices:
  * The HOST pre-normalizes the diffusion matrices and ships 64*A_norm^T in
    fp8e4m3 (the 64x keeps entries in e4m3's normal range; the 1/64 and
    1/4096 compensations are folded into the host-prepped projection
    weights).  No degree sums, reciprocals, or column scales on-chip.
  * All 8 diffusion hops are fp8 DoubleRow matmuls (two 128-row k-tiles per
    instruction, 0.5 cycles/row): z^T = act_nm.T @ an.  PSUM results are
    evacuated as plain bf16 copies (DVE chunk0 / ACT chunk1 in parallel).
  * bf16 z^T is PE-transposed back to node-major and evacuated to fp8 for
    the next hop's stationary.  Projections contract features with bf16
    stationary weights against bf16 feature-major activations.
  * Tail: w1u = sigmoid(-pre_u) so out = uxh + w1u*c takes two elementwise
    ops per chunk; uxh = u*xh is precomputed off the critical path.
"""

import numpy as np
import ml_dtypes

import concourse.bacc as bacc
import concourse.mybir as mybir
from concourse import tile
from concourse.bass_utils import run_bass_kernel_spmd

B, N, D = 8, 1024, 64
F = 2 * D       # 128 per-node features into gconv1
NT = N // 128   # 8 node tiles
O1, O2 = 2 * D, D
ASC = 64.0      # host prescale on the normalized adjacency

F32 = mybir.dt.float32
BF16 = mybir.dt.bfloat16
FP8 = mybir.dt.float8e4
S2 = float(1.0 / (64.0 * 64.0))   # 1/ASC^2 pre-activation rescale
NPBF = ml_dtypes.bfloat16
NPF8 = ml_dtypes.float8_e4m3
DR = mybir.MatmulPerfMode.DoubleRow
Copy = mybir.ActivationFunctionType.Copy
Sig = mybir.ActivationFunctionType.Sigmoid
Tanh = mybir.ActivationFunctionType.Tanh

FREE = 512
CH = [(i, min(i + FREE, N)) for i in range(0, N, FREE)]
TCH = [(i, min(i + 256, N)) for i in range(0, N, 256)]
# host-pre-summed stationary weights (feat-major), split so the gconv1
# block can be DMA'd between the a1 quads:
#   wpk1: wx1 wz10 wz11 wz20 wz21 (O1 each)
#   wpk2: vx1 vx1sh vz10 vz11 vz20 vz21 (O2 each)
WPK1 = 5 * O1
WPK2 = 6 * O2

_cache: dict = {}


def _build_nc():
    nc = bacc.Bacc("TRN2", target_bir_lowering=False, debug=False, num_devices=8)

    pk0_d = nc.declare_dram_parameter("pk0", [128, NT + 2, 128], FP8,
                                      isOutput=False)
    a0n_d = nc.declare_dram_parameter("a0n", [N, N], FP8, isOutput=False)
    a1n_d = nc.declare_dram_parameter("a1n", [N, N], FP8, isOutput=False)
    xct_d = nc.declare_dram_parameter("xct", [128, N], BF16, isOutput=False)
    wpk1_d = nc.declare_dram_parameter("wpk1", [128, WPK1], BF16,
                                       isOutput=False)
    wpk2_d = nc.declare_dram_parameter("wpk2", [128, WPK2], BF16,
                                       isOutput=False)
    fpk_d = nc.declare_dram_parameter("fpk", [128, 4], F32, isOutput=False)
    xht_d = nc.declare_dram_parameter("xht", [D, N], BF16, isOutput=False)
    out_d = nc.declare_dram_parameter("out", [D, N], BF16, isOutput=True)

    with tile.TileContext(nc) as tc:
        with (
            tc.tile_pool(name="sb", bufs=1) as sb,
            tc.tile_pool(name="zp", bufs=1) as zp,
            tc.tile_pool(name="nmp", bufs=1) as nmp,
            tc.tile_pool(name="ph", bufs=2, space="PSUM") as ph,
            tc.tile_pool(name="pp", bufs=1, space="PSUM") as pp,
            tc.tile_pool(name="pd", bufs=2, space="PSUM") as pd,
        ):
            # ---------------- input DMAs (SP queue, in need order) ----------
            pk0 = sb.tile([128, NT + 2, 128], FP8, tag="pk0")
            nc.sync.dma_start(pk0[:], pk0_d[:])
            a0sb = sb.tile([128, NT, N], FP8, tag="a0sb")
            a1sb = sb.tile([128, NT, N], FP8, tag="a1sb")
            wpk1 = sb.tile([128, WPK1], BF16, tag="wpk1")
            wpk2 = sb.tile([128, WPK2], BF16, tag="wpk2")
            qd = []
            for asb, ad in ((a0sb, a0n_d), (a1sb, a1n_d)):
                for p in range(2):
                    qd.append((asb[:, 4 * p : 4 * p + 4, :],
                               ad[512 * p : 512 * (p + 1), :].rearrange(
                                   "(t p) n -> p t n", p=128)))
            for k in (0, 1, 2):
                nc.sync.dma_start(*qd[k])
            nc.sync.dma_start(wpk1[:], wpk1_d[:])   # before the last a1 quad
            nc.sync.dma_start(*qd[3])
            xcT = sb.tile([128, N], BF16, tag="xcT")
            nc.sync.dma_start(xcT[:], xct_d[:])
            nc.sync.dma_start(wpk2[:], wpk2_d[:])
            fpk = sb.tile([128, 4], F32, tag="fpk")
            nc.sync.dma_start(fpk[:], fpk_d[:])
            xhT = sb.tile([D, N], BF16, tag="xhT")
            nc.sync.dma_start(xhT[:], xht_d[:])

            xc3 = pk0[:, 0:NT, :]            # node-major fp8 hop1 stationary
            # bf16 identity for PE transposes, byte-packed in the fp8 param
            identb = pk0[:, NT : NT + 2, :].rearrange(
                "p a b -> p (a b)").bitcast(BF16)

            # host-pre-summed weights / bias slices
            wx1, wz10, wz11, wz20, wz21 = (
                wpk1[:, i * O1 : (i + 1) * O1] for i in range(5))
            vx1, vx1sh, vz10, vz11, vz20, vz21 = (
                wpk2[:, i * O2 : (i + 1) * O2] for i in range(6))
            bias1 = fpk[:, 0:1]
            bias1n = fpk[:, 1:2]
            bias2 = fpk[0:O2, 2:3]

            # ---------------- early Pool work ------------------------------
            xc2nm = sb.tile([128, NT, 128], FP8, tag="xc2nm")
            nc.gpsimd.tensor_copy(xc2nm[:, :, 0:D], xc3[:, :, 0:D])

            # dummy sigmoid so the single act-table load (which also covers
            # Copy and Tanh) happens during the DMA-idle front, not at ~15us
            warm = sb.tile([1, 2], F32, tag="warm")
            nc.vector.memset(warm[0:1, 0:1], 0.0)
            nc.scalar.activation(warm[0:1, 1:2], warm[0:1, 0:1], Sig)

            # ---------------- helpers --------------------------------------
            def hop_chunk(pt, lhs3, asb, c0, c1, qs=range(NT // 2)):
                for q in qs:
                    nc.tensor.matmul(
                        pt[:], lhs3[:, 2 * q : 2 * q + 2, :],
                        asb[:, 2 * q : 2 * q + 2, c0:c1],
                        start=(q == 0), stop=(q == NT // 2 - 1),
                        perf_mode=DR)

            def hop(lhs3, asb, tag, qs=range(NT // 2), pts=None):
                outs = pts or [ph.tile([128, FREE], F32, tag="hopch",
                                       name=f"ph_{tag}_{c0}")
                               for c0, _ in CH]
                for pt, (c0, c1) in zip(outs, CH):
                    hop_chunk(pt, lhs3, asb, c0, c1, qs=qs)
                return outs

            def zevac(pts, tag, engines=(None, None)):
                """plain PSUM -> bf16 SBUF copies; DVE c0 / ACT c1 default."""
                z = zp.tile([128, N], BF16, tag=f"z_{tag}")
                for i, (c0, c1) in enumerate(CH):
                    eng = engines[i]
                    if eng is None:
                        eng = nc.vector if i == 0 else nc.scalar
                    if eng is nc.scalar:
                        eng.activation(z[:, c0:c1], pts[i][:], Copy)
                    else:
                        eng.tensor_copy(z[:, c0:c1], pts[i][:])
                return z

            def tr_group(zsrc, g, nm, parts=128, foff=0, evac=None):
                """PE-transpose 4 tiles of bf16 z -> evac to fp8 node-major."""
                ptr = pd.tile([128, 4, 128], BF16, tag="pdb", bufs=2)
                for k in range(4):
                    t = 4 * g + k
                    nc.tensor.transpose(
                        ptr[:, k, 0:parts],
                        zsrc[0:parts, t * 128 : (t + 1) * 128],
                        identb[0:parts, 0:parts])
                evac = evac or (nc.vector if g == 0 else nc.scalar)
                if evac is nc.scalar:
                    evac.activation(
                        nm[:, 4 * g : 4 * g + 4, foff : foff + parts],
                        ptr[:, :, 0:parts], Copy)
                else:
                    evac.tensor_copy(
                        nm[:, 4 * g : 4 * g + 4, foff : foff + parts],
                        ptr[:, :, 0:parts])

            def w_group(zsrc, g, wmv, o, nm, evac=None):
                """node-major W2 = z1 @ w2raw: multiply z1^T slices by the
                raw hop2 projection weights in the PE pass that used to be
                the identity transpose; evac fp8."""
                ptr = pd.tile([128, 4, 128], F32, tag="pdw")
                for k in range(4):
                    t = 4 * g + k
                    nc.tensor.matmul(
                        ptr[:, k, 0:o],
                        zsrc[:, t * 128 : (t + 1) * 128], wmv,
                        start=True, stop=True)
                evac = evac or (nc.vector if g == 0 else nc.scalar)
                if evac is nc.scalar:
                    evac.activation(nm[:, 4 * g : 4 * g + 4, :],
                                    ptr[:, :, 0:o], Copy)
                else:
                    evac.tensor_copy(nm[:, 4 * g : 4 * g + 4, :],
                                     ptr[:, :, 0:o])

            def hop_into(pre, lhs3, asb, chunks=CH, start=False, stop=False):
                """folded hop2: W2^T @ a accumulates straight into pre."""
                for c0, c1 in chunks:
                    for q in range(NT // 2):
                        nc.tensor.matmul(
                            pre[:, c0:c1], lhs3[:, 2 * q : 2 * q + 2, :],
                            asb[:, 2 * q : 2 * q + 2, c0:c1],
                            start=(start and q == 0),
                            stop=(stop and q == NT // 2 - 1),
                            perf_mode=DR)

            def proj(pre, w, rhs, start=False, stop=False, chunks=CH):
                for c0, c1 in chunks:
                    nc.tensor.matmul(pre[:, c0:c1], w, rhs[:, c0:c1],
                                     start=start, stop=stop)

            # ---------------- gconv1 ---------------------------------------
            # chain0 (moving a0n): hop1 -> z1 -> W2a = z1 @ wz20raw (fused
            # into the transpose pass) -> folded hop2 straight into pre1
            p1a = hop(xc3, a0sb, "1a")
            z1c0 = zevac(p1a, "1c0")
            W2a = nmp.tile([128, NT, O1], FP8, tag="W2a")
            pre1 = pp.tile([O1, N], F32, tag="pre")
            with tc.high_priority():
                w_group(z1c0, 0, wz20, O1, W2a)
                w_group(z1c0, 1, wz20, O1, W2a, evac=nc.scalar)
                proj(pre1, wz10, z1c0, start=True)
                hop_into(pre1, W2a, a0sb)
            # chain1 hop1 starts on a1's first quad
            p1b = hop(xc3, a1sb, "1b", qs=(0, 1))
            hop(xc3, a1sb, "1b", qs=(2, 3), pts=p1b)
            z1c1 = zevac(p1b, "1c1")
            W2b = nmp.tile([128, NT, O1], FP8, tag="W2b")
            w_group(z1c1, 0, wz21, O1, W2b)
            w_group(z1c1, 1, wz21, O1, W2b, evac=nc.scalar)
            proj(pre1, wx1, xcT)
            proj(pre1, wz11, z1c1)
            hop_into(pre1, W2b, a1sb, stop=True)

            # ---------------- sigmoid split / rxh --------------------------
            # pipelined per-256: sigmoid chunk -> rxh mul -> 2-tile PE
            # transposes -> fp8 evac into xc2nm's rxh half
            val_r = sb.tile([D, N], F32, tag="val_r")
            rxhT = sb.tile([D, N], BF16, tag="rxhT")
            for i, (c0, c1) in enumerate(CH):
                nc.scalar.activation(val_r[:, c0:c1], pre1[0:D, c0:c1], Sig,
                                     bias=bias1[0:D], scale=S2)
                for g in (2 * i, 2 * i + 1):
                    t0 = 256 * g
                    nc.vector.tensor_mul(rxhT[:, t0 : t0 + 256],
                                         val_r[:, t0 : t0 + 256],
                                         xhT[:, t0 : t0 + 256])
                    ptr = pd.tile([128, 2, 128], BF16, tag="pdb", bufs=2,
                                  name=f"rxtr{g}")
                    for k in range(2):
                        t = 2 * g + k
                        nc.tensor.transpose(
                            ptr[:, k, 0:D],
                            rxhT[0:D, t * 128 : (t + 1) * 128],
                            identb[0:D, 0:D])
                    evac = nc.vector if g % 2 == 0 else nc.scalar
                    if evac is nc.scalar:
                        evac.activation(xc2nm[:, 2 * g : 2 * g + 2, D:F],
                                        ptr[:, :, 0:D], Copy)
                    else:
                        evac.tensor_copy(xc2nm[:, 2 * g : 2 * g + 2, D:F],
                                         ptr[:, :, 0:D])

            # w1u = 1-u = sigmoid(-pre_u); out = (xh - w1u*xh) + w1u*c
            w1u = sb.tile([D, N], BF16, tag="w1u")
            nc.scalar.activation(w1u[:], pre1[D:F, :], Sig,
                                 bias=bias1n[D:F], scale=-S2)
            wxh = sb.tile([D, N], BF16, tag="wxh")
            uxh = sb.tile([D, N], BF16, tag="uxh")
            nc.gpsimd.tensor_mul(wxh[:], w1u[:], xhT[:])
            nc.gpsimd.tensor_sub(uxh[:], xhT[:], wxh[:])

            # ---------------- gconv2 ---------------------------------------
            p1a2 = hop(xc2nm, a0sb, "1a2")
            z1c0_2 = zevac(p1a2, "1c0_2")
            W2a2 = nmp.tile([128, NT, O2], FP8, tag="W2a2")
            w_group(z1c0_2, 0, vz20, O2, W2a2)
            w_group(z1c0_2, 1, vz20, O2, W2a2, evac=nc.scalar)
            p1b2 = hop(xc2nm, a1sb, "1b2")
            pre2 = pp.tile([O2, N], F32, tag="pre")
            # start=True writes must cover full 2KB PSUM zero-regions
            proj(pre2, vz10, z1c0_2, start=True, chunks=CH)
            hop_into(pre2, W2a2, a0sb, chunks=TCH)
            z1c1_2 = zevac(p1b2, "1c1_2")
            W2b2 = nmp.tile([128, NT, O2], FP8, tag="W2b2")
            w_group(z1c1_2, 0, vz21, O2, W2b2)
            w_group(z1c1_2, 1, vz21, O2, W2b2, evac=nc.scalar)
            proj(pre2, vx1[0:D], xcT[0:D, :], chunks=TCH)
            proj(pre2, vx1sh[0:D], rxhT, chunks=TCH)
            proj(pre2, vz11, z1c1_2, chunks=TCH)

            # folded last hop chunk-pipelined into the tail
            cT = sb.tile([D, N], BF16, tag="cT")
            t1 = sb.tile([D, N], BF16, tag="t1")
            outT = sb.tile([D, N], BF16, tag="outT")
            hop_into(pre2, W2b2, a1sb, chunks=TCH, stop=True)
            for i, (c0, c1) in enumerate(CH):
                nc.scalar.activation(cT[:, c0:c1], pre2[:, c0:c1], Tanh,
                                     bias=bias2, scale=S2)
                nc.vector.tensor_mul(t1[:, c0:c1], w1u[:, c0:c1],
                                     cT[:, c0:c1])
                nc.vector.tensor_add(outT[:, c0:c1], uxh[:, c0:c1],
                                     t1[:, c0:c1])
                nc.sync.dma_start(out_d[:, c0:c1], outT[:, c0:c1])

    nc.finalize()
    return nc


def _prep_inputs(inputs, hx, adj, W0, b0, W1, b1, Wc0, bc0, Wc1, bc1):
    """Host-side layout prep -> per-core input maps."""
    eye = np.eye(N, dtype=np.float32)
    identb = (np.eye(128, dtype=np.float32).astype(NPBF)
              .view(np.uint8).reshape(128, 2, 128).view(NPF8))

    def wblk(W, o):   # [o, 4F] -> 4 feat-major [128, o] blocks
        WT = np.ascontiguousarray(W.T)          # (F*4, o)
        return WT.reshape(128, 4, o).transpose(1, 0, 2)

    # pre1/pre2 are computed at ASC^2 scale (the folded hop2 contributions
    # carry ASC from each of the two adjacency passes); the activations
    # rescale by S2.  z1-projection weights carry one ASC, x-projection
    # weights two, hop2 weights stay raw (they multiply ASC-scaled z1).
    k1, k2 = np.float32(ASC), np.float32(ASC**2)
    w0m, w1m = wblk(W0, O1), wblk(W1, O1)
    wc0m, wc1m = wblk(Wc0, O2), wblk(Wc1, O2)
    vx1 = k2 * (wc0m[0] + wc1m[0])
    vx1sh = np.zeros_like(vx1)
    vx1sh[:D] = vx1[D:F]                        # rxh-half weights at base 0
    wpk1 = np.concatenate(
        [k2 * (w0m[0] + w1m[0]), k1 * (w0m[1] + w0m[2]),
         k1 * (w1m[1] + w1m[2]), w0m[3], w1m[3]], axis=1).astype(NPBF)
    wpk2 = np.concatenate(
        [vx1, vx1sh, k1 * (wc0m[1] + wc0m[2]), k1 * (wc1m[1] + wc1m[2]),
         wc0m[3], wc1m[3]], axis=1).astype(NPBF)
    fpk = np.zeros((128, 4), np.float32)
    fpk[:, 0] = b0 + b1
    fpk[:, 1] = -(b0 + b1)
    fpk[:O2, 2] = bc0 + bc1
    in_maps = []
    xi_all = inputs.reshape(B, N, D)
    xh_all = hx.reshape(B, N, D)
    for b in range(B):
        adjI = adj[b] + eye
        d0 = adjI.sum(axis=0)                   # A0 scale (colsum)
        d1 = adjI.sum(axis=1)                   # A1 scale (rowsum)
        a0n = (ASC * adjI.T / d0[None, :]).astype(NPF8)
        a1n = (ASC * adjI / d1[None, :]).astype(NPF8)
        xcb = np.concatenate([xi_all[b], xh_all[b]], axis=1)  # (N, F)
        xc_nm = xcb.reshape(NT, 128, F).transpose(1, 0, 2)    # [128, NT, F]
        pk0 = np.concatenate([xc_nm.astype(NPF8), identb], axis=1)
        m = {
            "pk0": np.ascontiguousarray(pk0),
            "a0n": np.ascontiguousarray(a0n),
            "a1n": np.ascontiguousarray(a1n),
            "xct": np.ascontiguousarray(xcb.T).astype(NPBF),
            "wpk1": wpk1,
            "wpk2": wpk2,
            "fpk": fpk,
            "xht": np.ascontiguousarray(xh_all[b].T).astype(NPBF),
        }
        in_maps.append(m)
    return in_maps


def kernel(**inputs) -> np.ndarray:
    args = {k: np.asarray(v) for k, v in inputs.items()}
    if "nc" not in _cache:
        _cache["nc"] = _build_nc()
    nc = _cache["nc"]
    in_maps = _prep_inputs(
        args["inputs"], args["hx"], args["adj"],
        args["W0"], args["b0"], args["W1"], args["b1"],
        args["Wc0"], args["bc0"], args["Wc1"], args["bc1"],
    )
    res = run_bass_kernel_spmd(nc, in_maps, list(range(B)))
    out = np.stack(
        [np.ascontiguousarray(
            res.results[b]["out"].astype(np.float32).T).reshape(N * D)
         for b in range(B)]
    )
    return out.astype(np.float32)
